# revision 3
# baseline (speedup 1.0000x reference)
"""Trainium2 Bass kernel for BP symmetry-function fingerprints (G2 radial + G4 angular).

Strategy (per the edge/data-parallel sharding hint):
  - Triplets are sharded contiguously across the 8 NeuronCores; pairs are sharded
    for the G2 part. Every core builds the full per-pair feature table on device
    (dense, cheap) since its triplet shard references arbitrary pairs.
  - Per-pair features [ux,uy,uz,g,dist,fc,lo,hi8] are stored in a DRAM table.
    g = fc*exp(-eta4*d^2) folds the (uniform) eta_g4 term; lo/hi8 presplit the
    central-atom index for the scatter stage.
  - Per-triplet pair rows are fetched with chained 128-row indirect DMAs (Pool).
  - G4 math is dense DVE/ACT work; powers of (1 +/- cos) by repeated squaring.
  - The segment-sum onto atoms uses a collision-safe PE path: per 128-triplet
    column, one-hot matrices over atom%128 (S) and atom//128 (M) are built with
    is_equal against iota constants, and S^T @ (M (x) sf) accumulates into a
    persistent [128, 157*8] PSUM region. No DMA read-modify-write anywhere.
  - Host work is only contiguous slicing/reshaping of inputs and the final sum
    of the 8 per-core partial fingerprints.
"""
import sys

sys.path.insert(0, "/opt/trn_rl_repo")

import numpy as np

N_ATOMS = 20000
N_PAIRS = 1_000_000
N_TRIP = 8_000_000
RC = 6.0
N_SF = 8

P = 128
NP_PAD = 1 << 20            # padded pair count (8192 cols x 128 partitions)
PCOLS = NP_PAD // P         # 8192
DUMMY = NP_PAD - 1          # dummy pair row (fc=0 -> contributes nothing)
NT_CORE = N_TRIP // 8       # triplets per core
TCOLS = 7936                # 62 chunks x 128 (>= NT_CORE/128 = 7812.5)
G2_PER_CORE = NP_PAD // 8   # 131072 pairs per core for G2
G2COLS = G2_PER_CORE // P   # 1024
QN = 157                    # ceil(20000/128) atom-high planes
ACC_W = QN * N_SF           # 1256

_CACHE = {}


def _build_program(Rs, eta_g2, lambd, zeta, eta_g4):
    import concourse.bass as bass
    import concourse.tile as tile
    from concourse import bacc, mybir

    f32 = mybir.dt.float32
    bf16 = mybir.dt.bfloat16
    i32 = mybir.dt.int32
    AF = mybir.ActivationFunctionType
    ALU = mybir.AluOpType

    eta4 = float(eta_g4[0])
    zints = [int(round(float(z))) for z in zeta]
    lsigns = [1 if float(l) >= 0 else -1 for l in lambd]
    coefs = [2.0 ** (1.0 - z) for z in zints]

    nc = bacc.Bacc("TRN2", target_bir_lowering=False, debug=False, num_devices=8)

    diff3_ap = nc.dram_tensor("diff3", [P, PCOLS, 3], f32, kind="ExternalInput").ap()
    rind_ap = nc.dram_tensor("rind", [P, PCOLS], i32, kind="ExternalInput").ap()
    ij_ap = nc.dram_tensor("ij", [P, TCOLS], i32, kind="ExternalInput").ap()
    ik_ap = nc.dram_tensor("ik", [P, TCOLS], i32, kind="ExternalInput").ap()
    g2diff_ap = nc.dram_tensor("g2diff", [P, G2COLS, 3], f32, kind="ExternalInput").ap()
    g2rind_ap = nc.dram_tensor("g2rind", [P, G2COLS], i32, kind="ExternalInput").ap()
    iota_lo_ap = nc.dram_tensor("iota_lo", [P, P], f32, kind="ExternalInput").ap()
    iota_hi_ap = nc.dram_tensor("iota_hi", [P, QN], f32, kind="ExternalInput").ap()
    fp4p_ap = nc.dram_tensor("fp4p", [P, ACC_W], f32, kind="ExternalOutput").ap()
    fp2p_ap = nc.dram_tensor("fp2p", [P, ACC_W], f32, kind="ExternalOutput").ap()

    table = nc.dram_tensor("table", [NP_PAD, 8], f32)
    table_ap = table.ap()
    table_pm = table_ap.rearrange("(p c) f -> p c f", p=P)

    PI = float(np.pi)

    def pair_features(nc, pool, dt3, rt, C):
        """Dense per-pair features from diff tile [P,C,3] + rind tile [P,C].
        Returns staging tile [P, C, 8] f32: ux,uy,uz,g,dist,fc,lo,hi8."""
        st = pool.tile([P, C, 8], f32, tag="stage")
        d2 = pool.tile([P, C], f32, tag="d2")
        tmp = pool.tile([P, C], f32, tag="tmp")
        nc.vector.tensor_tensor(out=d2[:], in0=dt3[:, :, 0], in1=dt3[:, :, 0], op=ALU.mult)
        nc.vector.tensor_tensor(out=tmp[:], in0=dt3[:, :, 1], in1=dt3[:, :, 1], op=ALU.mult)
        nc.vector.tensor_tensor(out=d2[:], in0=d2[:], in1=tmp[:], op=ALU.add)
        nc.vector.tensor_tensor(out=tmp[:], in0=dt3[:, :, 2], in1=dt3[:, :, 2], op=ALU.mult)
        nc.vector.tensor_tensor(out=d2[:], in0=d2[:], in1=tmp[:], op=ALU.add)
        dist = pool.tile([P, C], f32, tag="dist")
        nc.scalar.activation(dist[:], d2[:], AF.Sqrt)
        inv = pool.tile([P, C], f32, tag="inv")
        nc.vector.reciprocal(out=inv[:], in_=dist[:])
        # fc = 0.5*cos(pi*d/rc) + 0.5 = 1 - sin^2(pi*d/(2*rc)), masked to d < rc
        fc = pool.tile([P, C], f32, tag="fc")
        nc.scalar.activation(fc[:], dist[:], AF.Sin, scale=PI / (2.0 * RC))
        nc.vector.tensor_tensor(out=fc[:], in0=fc[:], in1=fc[:], op=ALU.mult)
        nc.vector.tensor_scalar(out=fc[:], in0=fc[:], scalar1=-1.0, scalar2=1.0,
                                op0=ALU.mult, op1=ALU.add)
        nc.vector.tensor_scalar(out=tmp[:], in0=dist[:], scalar1=float(RC), scalar2=None,
                                op0=ALU.is_lt)
        nc.vector.tensor_tensor(out=fc[:], in0=fc[:], in1=tmp[:], op=ALU.mult)
        # g = fc * exp(-eta4*d2)
        g = pool.tile([P, C], f32, tag="g")
        nc.scalar.activation(g[:], d2[:], AF.Exp, scale=-eta4)
        nc.vector.tensor_tensor(out=g[:], in0=g[:], in1=fc[:], op=ALU.mult)
        # atom split: lo = a & 127 ; hi8 = (a >> 7) * 8
        loi = pool.tile([P, C], i32, tag="loi")
        nc.vector.tensor_scalar(out=loi[:], in0=rt[:], scalar1=127, scalar2=None,
                                op0=ALU.bitwise_and)
        hii = pool.tile([P, C], i32, tag="hii")
        nc.vector.tensor_scalar(out=hii[:], in0=rt[:], scalar1=7, scalar2=None,
                                op0=ALU.arith_shift_right)
        nc.vector.tensor_scalar(out=hii[:], in0=hii[:], scalar1=3, scalar2=None,
                                op0=ALU.logical_shift_left)
        # assemble
        nc.vector.tensor_tensor(out=st[:, :, 0], in0=dt3[:, :, 0], in1=inv[:], op=ALU.mult)
        nc.vector.tensor_tensor(out=st[:, :, 1], in0=dt3[:, :, 1], in1=inv[:], op=ALU.mult)
        nc.vector.tensor_tensor(out=st[:, :, 2], in0=dt3[:, :, 2], in1=inv[:], op=ALU.mult)
        nc.vector.tensor_copy(out=st[:, :, 3], in_=g[:])
        nc.vector.tensor_copy(out=st[:, :, 4], in_=dist[:])
        nc.vector.tensor_copy(out=st[:, :, 5], in_=fc[:])
        nc.vector.tensor_copy(out=st[:, :, 6], in_=loi[:])
        nc.vector.tensor_copy(out=st[:, :, 7], in_=hii[:])
        return st

    def scatter_columns(nc, pools, lo_src, hi_src, sf, acc, iota_lo_t, iota_hi_t):
        """For 128 columns: S/M one-hot + 3 matmuls accumulating into acc PSUM.
        lo_src/hi_src: callables r -> [P,1] f32 AP; sf: [P,128,8] bf16 tile."""
        spool, mpool, rpool = pools
        for r in range(P):
            S = spool.tile([P, P], bf16, tag="S")
            nc.vector.tensor_tensor(out=S[:], in0=lo_src(r).to_broadcast([P, P]),
                                    in1=iota_lo_t[:], op=ALU.is_equal)
            M = mpool.tile([P, QN], bf16, tag="M")
            nc.vector.tensor_tensor(out=M[:], in0=hi_src(r).to_broadcast([P, QN]),
                                    in1=iota_hi_t[:], op=ALU.is_equal)
            R = rpool.tile([P, ACC_W], bf16, tag="R")
            for s in range(N_SF):
                eng = nc.vector if s % 2 == 0 else nc.scalar
                if s % 2 == 0:
                    nc.vector.tensor_scalar(
                        out=R[:, s::N_SF], in0=M[:], scalar1=sf[:, r, s:s + 1],
                        scalar2=None, op0=ALU.mult)
                else:
                    nc.scalar.mul(R[:, s::N_SF], M[:], sf[:, r, s:s + 1])
            nc.tensor.matmul(out=acc[:, 0:512], lhsT=S[:], rhs=R[:, 0:512],
                             start=False, stop=True)
            nc.tensor.matmul(out=acc[:, 512:1024], lhsT=S[:], rhs=R[:, 512:1024],
                             start=False, stop=True)
            nc.tensor.matmul(out=acc[:, 1024:ACC_W], lhsT=S[:], rhs=R[:, 1024:ACC_W],
                             start=False, stop=True)

    with tile.TileContext(nc) as tc:
        with (
            tc.tile_pool(name="consts", bufs=1) as cpool,
            tc.tile_pool(name="build", bufs=2) as bpool,
            tc.tile_pool(name="idx", bufs=2) as ipool,
            tc.tile_pool(name="gath", bufs=2) as gpool,
            tc.tile_pool(name="math", bufs=2) as mpool_,
            tc.tile_pool(name="sf", bufs=2) as sfpool,
            tc.tile_pool(name="scol", bufs=3) as spool,
            tc.tile_pool(name="mcol", bufs=3) as mcpool,
            tc.tile_pool(name="rcol", bufs=3) as rpool,
            tc.tile_pool(name="evac", bufs=1) as epool,
            tc.tile_pool(name="psum", bufs=1, space="PSUM") as ppool,
        ):
            iota_lo_t = cpool.tile([P, P], f32)
            nc.sync.dma_start(iota_lo_t[:], iota_lo_ap[:])
            iota_hi_t = cpool.tile([P, QN], f32)
            nc.sync.dma_start(iota_hi_t[:], iota_hi_ap[:])

            # ---- Phase 1: build pair table (dense)
            with tc.For_i(0, PCOLS, P) as i:
                dt3 = bpool.tile([P, P, 3], f32, tag="dt3")
                nc.sync.dma_start(dt3[:], diff3_ap[:, bass.ds(i, P), :])
                rt = bpool.tile([P, P], i32, tag="rt")
                nc.sync.dma_start(rt[:], rind_ap[:, bass.ds(i, P)])
                st = pair_features(nc, bpool, dt3, rt, P)
                nc.sync.dma_start(table_pm[:, bass.ds(i, P), :], st[:])

            # ---- Phase 2: G4 triplets
            acc = ppool.tile([P, ACC_W], f32, space="PSUM")
            nc.vector.memset(acc[:], 0.0)
            with tc.For_i(0, TCOLS, P) as i:
                ijt = ipool.tile([P, P], i32, tag="ijt")
                nc.sync.dma_start(ijt[:], ij_ap[:, bass.ds(i, P)])
                ikt = ipool.tile([P, P], i32, tag="ikt")
                nc.sync.dma_start(ikt[:], ik_ap[:, bass.ds(i, P)])
                gij = gpool.tile([P, P, 8], f32, tag="gij")
                gik = gpool.tile([P, P, 4], f32, tag="gik")
                for r in range(P):
                    nc.gpsimd.indirect_dma_start(
                        out=gij[:, r, :], out_offset=None, in_=table_ap[:],
                        in_offset=bass.IndirectOffsetOnAxis(ap=ijt[:, r:r + 1], axis=0))
                for r in range(P):
                    nc.gpsimd.indirect_dma_start(
                        out=gik[:, r, :], out_offset=None, in_=table_ap[:],
                        in_offset=bass.IndirectOffsetOnAxis(ap=ikt[:, r:r + 1], axis=0))
                # dense G4 math
                c = mpool_.tile([P, P], f32, tag="c")
                t1 = mpool_.tile([P, P], f32, tag="t1")
                nc.vector.tensor_tensor(out=c[:], in0=gij[:, :, 0], in1=gik[:, :, 0], op=ALU.mult)
                nc.vector.tensor_tensor(out=t1[:], in0=gij[:, :, 1], in1=gik[:, :, 1], op=ALU.mult)
                nc.vector.tensor_tensor(out=c[:], in0=c[:], in1=t1[:], op=ALU.add)
                nc.vector.tensor_tensor(out=t1[:], in0=gij[:, :, 2], in1=gik[:, :, 2], op=ALU.mult)
                nc.vector.tensor_tensor(out=c[:], in0=c[:], in1=t1[:], op=ALU.add)
                gg = mpool_.tile([P, P], f32, tag="gg")
                nc.vector.tensor_tensor(out=gg[:], in0=gij[:, :, 3], in1=gik[:, :, 3], op=ALU.mult)
                # powers of (1 + c) and (1 - c)
                pw = {}
                need = set()
                for z, ls in zip(zints, lsigns):
                    k = 1
                    while k <= z:
                        need.add((ls, k))
                        k *= 2
                for sgn in (1, -1):
                    if not any(s == sgn for s, _ in need):
                        continue
                    b = mpool_.tile([P, P], f32, tag=f"b{sgn}")
                    if sgn == 1:
                        nc.vector.tensor_scalar(out=b[:], in0=c[:], scalar1=1.0,
                                                scalar2=None, op0=ALU.add)
                    else:
                        nc.vector.tensor_scalar(out=b[:], in0=c[:], scalar1=-1.0,
                                                scalar2=1.0, op0=ALU.mult, op1=ALU.add)
                    pw[(sgn, 1)] = b
                    k = 2
                    while any(s == sgn and kk >= k for s, kk in need):
                        bb = mpool_.tile([P, P], f32, tag=f"b{sgn}_{k}")
                        nc.vector.tensor_tensor(out=bb[:], in0=pw[(sgn, k // 2)][:],
                                                in1=pw[(sgn, k // 2)][:], op=ALU.mult)
                        pw[(sgn, k)] = bb
                        k *= 2
                # scaled gg per distinct coef
                ggc = {}
                for cf in sorted(set(coefs)):
                    if cf == 1.0:
                        ggc[cf] = gg
                    else:
                        t = mpool_.tile([P, P], f32, tag=f"gg{cf}")
                        nc.vector.tensor_scalar(out=t[:], in0=gg[:], scalar1=float(cf),
                                                scalar2=None, op0=ALU.mult)
                        ggc[cf] = t
                sf4 = sfpool.tile([P, P, N_SF], f32, tag="sf4")
                for s in range(N_SF):
                    nc.vector.tensor_tensor(out=sf4[:, :, s], in0=pw[(lsigns[s], zints[s])][:],
                                            in1=ggc[coefs[s]][:], op=ALU.mult)
                scatter_columns(nc, (spool, mcpool, rpool),
                                lambda r: gij[:, r, 6:7], lambda r: gij[:, r, 7:8],
                                sf4, acc, iota_lo_t, iota_hi_t)
            ev4 = epool.tile([P, ACC_W], f32)
            nc.vector.tensor_copy(out=ev4[:], in_=acc[:])
            nc.sync.dma_start(fp4p_ap[:], ev4[:])

            # ---- Phase 3: G2 pairs (this core's shard)
            acc2 = ppool.tile([P, ACC_W], f32, space="PSUM", tag="acc2")
            nc.vector.memset(acc2[:], 0.0)
            with tc.For_i(0, G2COLS, P) as i:
                dt3 = bpool.tile([P, P, 3], f32, tag="dt3")
                nc.sync.dma_start(dt3[:], g2diff_ap[:, bass.ds(i, P), :])
                rt = bpool.tile([P, P], i32, tag="rt")
                nc.sync.dma_start(rt[:], g2rind_ap[:, bass.ds(i, P)])
                st = pair_features(nc, bpool, dt3, rt, P)
                sf2 = sfpool.tile([P, P, N_SF], f32, tag="sf2")
                t1 = mpool_.tile([P, P], f32, tag="g2t1")
                t2 = mpool_.tile([P, P], f32, tag="g2t2")
                for s in range(N_SF):
                    nc.vector.tensor_scalar(out=t1[:], in0=st[:, :, 4], scalar1=-float(Rs[s]),
                                            scalar2=None, op0=ALU.add)
                    nc.vector.tensor_tensor(out=t2[:], in0=t1[:], in1=t1[:], op=ALU.mult)
                    nc.scalar.activation(t2[:], t2[:], AF.Exp, scale=-float(eta_g2[s]))
                    nc.vector.tensor_tensor(out=sf2[:, :, s], in0=t2[:], in1=st[:, :, 5],
                                            op=ALU.mult)
                scatter_columns(nc, (spool, mcpool, rpool),
                                lambda r: st[:, r, 6:7], lambda r: st[:, r, 7:8],
                                sf2, acc2, iota_lo_t, iota_hi_t)
            ev2 = epool.tile([P, ACC_W], f32, tag="ev2")
            nc.vector.tensor_copy(out=ev2[:], in_=acc2[:])
            nc.sync.dma_start(fp2p_ap[:], ev2[:])

    nc.compile()
    return nc


def kernel(diff, elems, ind_2, ind_3, Rs, eta_g2, lambd, zeta, eta_g4):
    from concourse.bass_utils import run_bass_kernel_spmd

    diff = np.asarray(diff, np.float32)
    ind_2 = np.asarray(ind_2, np.int32)
    ind_3 = np.asarray(ind_3, np.int32)
    Rs = np.asarray(Rs, np.float32)
    eta_g2 = np.asarray(eta_g2, np.float32)
    lambd = np.asarray(lambd, np.float32)
    zeta = np.asarray(zeta, np.float32)
    eta_g4 = np.asarray(eta_g4, np.float32)

    key = (tuple(Rs.tolist()), tuple(eta_g2.tolist()), tuple(lambd.tolist()),
           tuple(zeta.tolist()), tuple(eta_g4.tolist()))
    if key not in _CACHE:
        _CACHE[key] = _build_program(Rs, eta_g2, lambd, zeta, eta_g4)
    nc = _CACHE[key]

    # ---- host-side sharding (contiguous slices / reshapes only)
    n_pairs = diff.shape[0]
    diff_pad = np.empty((NP_PAD, 3), np.float32)
    diff_pad[:n_pairs] = diff
    diff_pad[n_pairs:] = (10.0, 0.0, 0.0)         # dummy rows: dist>rc -> fc=0
    rind_pad = np.zeros(NP_PAD, np.int32)
    rind_pad[:n_pairs] = ind_2[:, 0]
    diff3 = diff_pad.reshape(P, PCOLS, 3)
    rind = rind_pad.reshape(P, PCOLS)

    iota_lo = np.broadcast_to(np.arange(P, dtype=np.float32), (P, P)).copy()
    iota_hi = np.broadcast_to(np.arange(QN, dtype=np.float32) * 8.0, (P, QN)).copy()

    n_trip = ind_3.shape[0]
    per_core = n_trip // 8
    in_maps = []
    for ccc in range(8):
        sl = ind_3[ccc * per_core:(ccc + 1) * per_core]
        ijp = np.full(TCOLS * P, DUMMY, np.int32)
        ikp = np.full(TCOLS * P, DUMMY, np.int32)
        ijp[:per_core] = sl[:, 0]
        ikp[:per_core] = sl[:, 1]
        ij_arr = ijp.reshape(TCOLS, P).T.copy()
        ik_arr = ikp.reshape(TCOLS, P).T.copy()
        g2d = diff_pad[ccc * G2_PER_CORE:(ccc + 1) * G2_PER_CORE].reshape(P, G2COLS, 3)
        g2r = rind_pad[ccc * G2_PER_CORE:(ccc + 1) * G2_PER_CORE].reshape(P, G2COLS)
        in_maps.append(dict(diff3=diff3, rind=rind, ij=ij_arr, ik=ik_arr,
                            g2diff=np.ascontiguousarray(g2d),
                            g2rind=np.ascontiguousarray(g2r),
                            iota_lo=iota_lo, iota_hi=iota_hi))

    res = run_bass_kernel_spmd(nc, in_maps, list(range(8)))

    fp2 = np.zeros((QN * P, N_SF), np.float64)
    fp4 = np.zeros((QN * P, N_SF), np.float64)
    for ccc in range(8):
        r2 = res.results[ccc]["fp2p"].reshape(P, QN, N_SF)
        r4 = res.results[ccc]["fp4p"].reshape(P, QN, N_SF)
        fp2 += r2.transpose(1, 0, 2).reshape(QN * P, N_SF)
        fp4 += r4.transpose(1, 0, 2).reshape(QN * P, N_SF)
    n_atoms = N_ATOMS
    out = np.concatenate([fp2[:n_atoms], fp4[:n_atoms]], axis=1).astype(np.float32)
    return out
